# revision 34
# baseline (speedup 1.0000x reference)
"""CrossDepthAttention Trainium2 kernel.

Computation (per token t):
    q = x_t @ Wq.T                        (D,)
    k_n = h_{t,n} @ Wk.T, v_n = h_{t,n} @ Wv.T   for n in 0..7
    logits[h,n] = <q_h, k_{n,h}> / sqrt(Hd)
    attn = softmax_n(logits)
    out_h = sum_n attn[h,n] * v_{n,h}
    y_t = x_t + out @ Wo.T

Sharding: pure data-parallel over the B*S = 8192 tokens, 1024 tokens per
core on 8 cores (every token attends only to its own history, so there
is no cross-token coupling and no collective is needed).

Per-core kernel: token tiles of 128.  All projections run on the tensor
engine with fp32 PSUM accumulation; the k/v projections (16/18 of the
FLOPs) use fp8-e4m3 operands with perf_mode=DoubleRow (2 fp8 weights
per PE cell, 256-wide contraction per matmul, ~1.4x bf16 throughput);
q and o projections stay bf16.  The stationary operand is the
(transposed) activation tile and the moving operand is the (transposed)
weight, which yields outputs directly in token-on-partition layout.  The tiny per-token attention (N_prev=8) runs
on the vector engine with stride-0 broadcast APs; exp runs on the scalar
engine.  The attention output is transposed 128x128 on the tensor engine
to feed the output projection; the residual is added in fp32.
"""

import numpy as np
import ml_dtypes
from contextlib import ExitStack

import concourse.bass as bass
import concourse.mybir as mybir
import concourse.tile as tile
from concourse import bacc
from concourse.masks import make_identity

BF16 = mybir.dt.bfloat16
F8 = mybir.dt.float8e4
F32 = mybir.dt.float32

B, S, D = 4, 2048, 1024
NPREV = 8
H = 16
HD = D // H  # 64
BS = B * S
N_CORES = 8
T_CORE = BS // N_CORES  # 1024 tokens per core
P = 128  # partition / token-tile size
C = D // P  # 8 contraction chunks
C2 = C // 2  # 4 fp8 DoubleRow chunks (256-wide contraction each)
N_TILES = T_CORE // P  # 8 token tiles per core
HALF = 512  # matmul free-dim (one PSUM bank)
SCALE = 1.0 / float(np.sqrt(HD))
# fp8 scaling: history x16, Wk/Wv x64 keep e4m3 quantization in the normal
# range; the 16*64 factor is descaled via the exp() scale (k path) and a
# host-side 1/1024 fold into Wo (v path).
H_SCALE = 16.0
W_SCALE = 64.0
KV_DESCALE = 1.0 / (H_SCALE * W_SCALE)
DR = mybir.MatmulPerfMode.DoubleRow

_CACHE = {}


def build_program(
    n_tiles=N_TILES,
    repeat=1,
    attention=True,
    dma_once=False,
    no_tp=False,
    mm_only=False,
):
    """Build the single-core Bass/Tile program (run SPMD on 8 cores).

    repeat>1 wraps the whole computation in an on-device For_i loop —
    used only for timing (amortizes host dispatch overhead away).
    attention=False drops the DVE/ACT attention chain (timing diagnostic
    only — wrong numerics).
    """
    nc = bacc.Bacc("TRN2", debug=False, num_devices=N_CORES)
    t_tok = n_tiles * P

    curP = nc.dram_tensor("curP", [n_tiles, P, C2, 2, P], F8, kind="ExternalInput").ap()
    histP = nc.dram_tensor(
        "histP", [n_tiles, NPREV, P, C2, 2, P], F8, kind="ExternalInput"
    ).ap()
    xres = nc.dram_tensor("xres", [t_tok, D], F32, kind="ExternalInput").ap()
    wq = nc.dram_tensor("wqT", [C2, P, 2, D], F8, kind="ExternalInput").ap()
    wk = nc.dram_tensor("wkT", [C2, P, 2, D], F8, kind="ExternalInput").ap()
    wv = nc.dram_tensor("wvT", [C2, P, 2, D], F8, kind="ExternalInput").ap()
    wo = nc.dram_tensor("woT", [C2, P, 2, D], F8, kind="ExternalInput").ap()
    y = nc.dram_tensor("y", [t_tok, D], F32, kind="ExternalOutput").ap()

    with ExitStack() as ctx:
        tc = ctx.enter_context(tile.TileContext(nc))

        wpool = ctx.enter_context(tc.tile_pool(name="wpool", bufs=1))
        inpool = ctx.enter_context(tc.tile_pool(name="inpool", bufs=2))
        qpool = ctx.enter_context(tc.tile_pool(name="qpool", bufs=2))
        prodpool = ctx.enter_context(tc.tile_pool(name="prodpool", bufs=4))
        spool = ctx.enter_context(tc.tile_pool(name="spool", bufs=2))
        accpool = ctx.enter_context(tc.tile_pool(name="accpool", bufs=2))
        aopool = ctx.enter_context(tc.tile_pool(name="aopool", bufs=2))
        atpool = ctx.enter_context(tc.tile_pool(name="atpool", bufs=2))
        ypool = ctx.enter_context(tc.tile_pool(name="ypool", bufs=2))
        # phase_a (q/k/v) and phase_b (tp/o) get separate PSUM pools of
        # 1-bank [P, HALF] tiles so the k/v matmul stream never waits on
        # the attention tail through PSUM slot reuse.  6 + 2 = 8 banks.
        psum = ctx.enter_context(tc.tile_pool(name="psum", bufs=6, space="PSUM"))
        psum_b = ctx.enter_context(tc.tile_pool(name="psum_b", bufs=2, space="PSUM"))

        # Resident weights: w_sb[name][p, c2, i, dout] = W?T[c2*256+i*128+p, dout] * W_SCALE
        w_sb = {}
        for name, dram in (("wq", wq), ("wk", wk), ("wv", wv), ("wo", wo)):
            wt = wpool.tile([P, C2, 2, D], F8, name=f"{name}_sb", tag=name)
            nc.sync.dma_start(wt[:], dram.rearrange("c p i d -> p c i d"))
            w_sb[name] = wt
        ident = wpool.tile([P, P], BF16, name="ident", tag="ident")
        make_identity(nc, ident[:])
        if no_tp:
            ident8 = wpool.tile([P, P], F8, name="ident8", tag="ident8")
            nc.scalar.copy(ident8[:], ident[:])

        # state carried from phase A (projections+attention) to phase B
        # (output projection), software-pipelined one tile deep so the
        # tensor engine never waits on the vector engine.
        state = [None] * n_tiles

        dma_cache = {}

        def phase_a(it):
            tok0 = it * P
            if dma_once and it > 0:
                xt, ht, xr = dma_cache[0]
            else:
                xt = inpool.tile([P, C2, 2, P], F8, tag="xt", name="xt")
                nc.sync.dma_start(xt[:], curP[it])
                ht = inpool.tile([P, NPREV, C2, 2, P], F8, tag="ht", name="ht")
                nc.sync.dma_start(ht[:], histP[it].rearrange("n p c i t -> p n c i t"))
                xr = inpool.tile([P, D], F32, tag="xr", name="xr")
                nc.sync.dma_start(xr[:], xres[tok0 : tok0 + P, :])
                dma_cache[0] = (xt, ht, xr)

            # q projection: q[t, dout] accumulated over 4 DoubleRow din chunks
            qps = [psum.tile([P, HALF], F32, tag="mm", name="qp") for _ in range(2)]
            if not mm_only:
                for c in range(C2):
                    for j in range(2):
                        nc.tensor.matmul(
                            qps[j][:],
                            lhsT=xt[:, c],
                            rhs=w_sb["wq"][:, c, :, j * HALF : (j + 1) * HALF],
                            start=(c == 0),
                            stop=(c == C2 - 1),
                            perf_mode=DR,
                        )
                # descale the fp8 q path during the PSUM->SBUF copy
                q_sb = qpool.tile([P, D], F32, tag="q", name="q_sb")
                for j in range(2):
                    nc.scalar.mul(
                        q_sb[:, j * HALF : (j + 1) * HALF], qps[j][:], KV_DESCALE
                    )

            expt = spool.tile([P, NPREV, H], F32, tag="expt", name="expt")
            ssum = spool.tile([P, H], F32, tag="ssum", name="ssum")
            rsum = spool.tile([P, H], F32, tag="rsum", name="rsum")
            acc = accpool.tile([P, H, HD], F32, tag="acc", name="acc")
            HH = H // 2  # heads per half

            for n in range(NPREV):
                # 1-bank psum tiles: [kp|vp] x [heads 0-7 | heads 8-15]
                kps = [psum.tile([P, HALF], F32, tag="mm", name="kp") for _ in range(2)]
                vps = [psum.tile([P, HALF], F32, tag="mm", name="vp") for _ in range(2)]
                for c in range(C2):
                    st = ht[:, n, c]  # [P, 2, P] fp8 DoubleRow stationary
                    for j in range(2):
                        nc.tensor.matmul(
                            kps[j][:],
                            lhsT=st,
                            rhs=w_sb["wk"][:, c, :, j * HALF : (j + 1) * HALF],
                            start=(c == 0),
                            stop=(c == C2 - 1),
                            perf_mode=DR,
                        )
                        nc.tensor.matmul(
                            vps[j][:],
                            lhsT=st,
                            rhs=w_sb["wv"][:, c, :, j * HALF : (j + 1) * HALF],
                            start=(c == 0),
                            stop=(c == C2 - 1),
                            perf_mode=DR,
                        )

                if not attention or mm_only:
                    continue
                # logits_n[t, h] = sum_e q[t,h,e] * k_n[t,h,e], per half
                logit_n = spool.tile([P, H], F32, tag="logit", name="logit_n")
                for j in range(2):
                    prod = prodpool.tile([P, HH, HD], BF16, tag="prod", name="prod")
                    nc.vector.tensor_mul(
                        prod[:],
                        q_sb[:, j * HALF : (j + 1) * HALF].rearrange(
                            "p (h e) -> p h e", e=HD
                        ),
                        kps[j].rearrange("p (h e) -> p h e", e=HD),
                    )
                    nc.vector.reduce_sum(
                        logit_n[:, j * HH : (j + 1) * HH],
                        prod[:],
                        axis=mybir.AxisListType.X,
                    )
                # exp(scale * logits) on the scalar engine
                nc.scalar.activation(
                    expt[:, n, :],
                    logit_n[:],
                    mybir.ActivationFunctionType.Exp,
                    scale=SCALE * KV_DESCALE,
                )
                # acc += exp_n (broadcast over hd) * v_n, per half
                for j in range(2):
                    e_ap = expt[:, n, j * HH : (j + 1) * HH]
                    e_b = bass.AP(e_ap.tensor, e_ap.offset, e_ap.ap + [[0, HD]])
                    v_v = vps[j].rearrange("p (h e) -> p h e", e=HD)
                    acc_h = acc[:, j * HH : (j + 1) * HH, :]
                    if n == 0:
                        nc.vector.tensor_mul(acc_h, v_v, e_b)
                    else:
                        avt = prodpool.tile(
                            [P, HH, HD], F32, tag="avt", name="avt"
                        )
                        nc.vector.tensor_mul(avt[:], v_v, e_b)
                        nc.vector.tensor_add(acc_h, acc_h, avt[:])

            if mm_only:
                state[it] = None
                return
            aout = aopool.tile([P, D], BF16, tag="aout", name="aout")
            if attention:
                # softmax denominator and normalization (+ cast to bf16)
                nc.vector.reduce_sum(
                    ssum[:], expt.rearrange("p n h -> p h n"), axis=mybir.AxisListType.X
                )
                nc.vector.reciprocal(rsum[:], ssum[:])
                r_ap = rsum[:]
                r_b = bass.AP(r_ap.tensor, r_ap.offset, r_ap.ap + [[0, HD]])
                nc.vector.tensor_mul(
                    aout.rearrange("p (h e) -> p h e", e=HD), acc[:], r_b
                )
            else:
                nc.scalar.copy(aout[:, :HALF], qps[0][:])
                nc.scalar.copy(aout[:, HALF:], qps[1][:])
            state[it] = (aout, xr, tok0)

        def phase_b(it):
            aout, xr, tok0 = state[it]
            state[it] = None
            # transpose attention output 128x128 on the tensor engine;
            # the PSUM->SBUF copy converts to fp8 with a 1/64 rescale
            # (aout carries x1024 from the fp8 v path -> aoutT carries x16)
            if no_tp:
                ia = ident8[:]
                aoutT = bass.AP(
                    ia.tensor, ia.offset, [ia.ap[0], [0, C2], [0, 2], ia.ap[1]]
                )
            else:
                aoutT = atpool.tile([P, C2, 2, P], F8, tag="aoutT", name="aoutT")
                for g in range(2):
                    tp = psum_b.tile([P, 4 * P], BF16, tag="tp", name="tp")
                    for cc in range(4):
                        c = g * 4 + cc
                        nc.tensor.transpose(
                            tp[:, cc * P : (cc + 1) * P],
                            aout[:, c * P : (c + 1) * P],
                            ident[:],
                        )
                    nc.scalar.mul(
                        aoutT[:, g * 2 : (g + 1) * 2].rearrange("p c i t -> p (c i t)"),
                        tp[:],
                        1.0 / 64.0,
                    )
            # output projection (fp8 DoubleRow) + descale + residual
            y_sb = ypool.tile([P, D], F32, tag="ysb", name="y_sb")
            yps = [psum_b.tile([P, HALF], F32, tag="tp", name="yp") for _ in range(2)]
            for c in range(C2):
                for j in range(2):
                    nc.tensor.matmul(
                        yps[j][:],
                        lhsT=aoutT[:, c],
                        rhs=w_sb["wo"][:, c, :, j * HALF : (j + 1) * HALF],
                        start=(c == 0),
                        stop=(c == C2 - 1),
                        perf_mode=DR,
                    )
            for j in range(2):
                nc.scalar.mul(y_sb[:, j * HALF : (j + 1) * HALF], yps[j][:], KV_DESCALE)
            nc.vector.tensor_add(y_sb[:], y_sb[:], xr[:])
            nc.sync.dma_start(y[tok0 : tok0 + P, :], y_sb[:])

        def whole_body():
            for it in range(n_tiles + 1):
                if it < n_tiles:
                    phase_a(it)
                if it >= 1 and not mm_only:
                    phase_b(it - 1)

        if repeat == 1:
            whole_body()
        else:
            with tc.For_i(0, repeat, 1):
                whole_body()

    nc.compile()
    return nc


def prep_inputs(current, history, Wq, Wk, Wv, Wo, n_cores=N_CORES):
    """Host-side shard + layout prep.  Returns per-core input maps."""
    bf16 = ml_dtypes.bfloat16
    f8 = ml_dtypes.float8_e4m3  # TRN FP8_EXP4-compatible (max +-240)
    cur = np.ascontiguousarray(current.reshape(BS, D)).astype(np.float32)
    hist = history.reshape(BS, NPREV, D)

    n_tiles_total = BS // P
    # curP[tile, p, c2, i, t] = cur[tile*128 + t, c2*256 + i*128 + p] * 16
    cur_f8 = (cur * H_SCALE).astype(f8)
    curP = np.ascontiguousarray(
        cur_f8.reshape(n_tiles_total, P, C2, 2, P).transpose(0, 4, 2, 3, 1)
    )
    # histP[tile, n, p, c2, i, t] = hist[tile*128 + t, n, c2*256 + i*128 + p] * 16
    hist_f8 = (hist.astype(np.float32) * H_SCALE).astype(f8)
    histP = np.ascontiguousarray(
        hist_f8.reshape(n_tiles_total, P, NPREV, C2, 2, P).transpose(0, 2, 5, 3, 4, 1)
    )

    def wprep8(w):
        # w?T8[c2, p, i, dout] = W.T[c2*256 + i*128 + p, dout] * 64
        wt = (w.T.astype(np.float32) * W_SCALE).astype(f8)
        return np.ascontiguousarray(wt.reshape(C2, 2, P, D).transpose(0, 2, 1, 3))

    wqT, wkT, wvT, woT = (wprep8(w) for w in (Wq, Wk, Wv, Wo))

    tiles_per_core = n_tiles_total // n_cores
    in_maps = []
    for ci in range(n_cores):
        t0 = ci * tiles_per_core
        sl = slice(ci * T_CORE, (ci + 1) * T_CORE)
        in_maps.append(
            {
                "curP": np.ascontiguousarray(curP[t0 : t0 + tiles_per_core]),
                "histP": np.ascontiguousarray(histP[t0 : t0 + tiles_per_core]),
                "xres": np.ascontiguousarray(cur[sl]),
                "wqT": wqT,
                "wkT": wkT,
                "wvT": wvT,
                "woT": woT,
            }
        )
    return in_maps


def kernel(current, history, Wq, Wk, Wv, Wo):
    from concourse.bass_utils import run_bass_kernel_spmd

    if "nc" not in _CACHE:
        _CACHE["nc"] = build_program(N_TILES)
    nc = _CACHE["nc"]

    in_maps = prep_inputs(current, history, Wq, Wk, Wv, Wo)
    results = run_bass_kernel_spmd(nc, in_maps, core_ids=list(range(N_CORES))).results
    y = np.concatenate([results[ci]["y"] for ci in range(N_CORES)], axis=0)
    return y.reshape(B, S, D).astype(np.float32)



# revision 38
# speedup vs baseline: 1.0764x; 1.0764x over previous
"""CrossDepthAttention Trainium2 kernel.

Computation (per token t):
    q = x_t @ Wq.T                        (D,)
    k_n = h_{t,n} @ Wk.T, v_n = h_{t,n} @ Wv.T   for n in 0..7
    logits[h,n] = <q_h, k_{n,h}> / sqrt(Hd)
    attn = softmax_n(logits)
    out_h = sum_n attn[h,n] * v_{n,h}
    y_t = x_t + out @ Wo.T

Sharding: pure data-parallel over the B*S = 8192 tokens, 1024 tokens per
core on 8 cores (every token attends only to its own history, so there
is no cross-token coupling and no collective is needed).

Per-core kernel: token tiles of 128.  All four projections (q/k/v/o)
run on the tensor engine as fp8-e4m3 matmuls with perf_mode=DoubleRow
(2 fp8 weights per PE cell, 256-wide contraction per matmul, ~1.4x bf16
throughput) accumulating in fp32 PSUM.  Activations are scaled x16 and
weights x64 on the host to keep e4m3 quantization in the normal range;
the x1024 product scale is descaled for free in existing ACT copies
(q path), the exp() scale constant (k path), and a 1/64 rescale in the
attention-output transpose copy plus a final 1/1024 ACT mul (v/o path).
The stationary operand is the (transposed) activation tile and the
moving operand is the (transposed) weight, which yields outputs
directly in token-on-partition layout.  End-to-end rel err ~1.03e-2
(gate 2e-2), dominated by fp8 input quantization.  The tiny per-token attention (N_prev=8) runs
on the vector engine with stride-0 broadcast APs; exp runs on the scalar
engine.  The attention output is transposed 128x128 on the tensor engine
to feed the output projection; the residual is added in fp32.
"""

import numpy as np
import ml_dtypes
from contextlib import ExitStack

import concourse.bass as bass
import concourse.mybir as mybir
import concourse.tile as tile
from concourse import bacc
from concourse.masks import make_identity

BF16 = mybir.dt.bfloat16
F8 = mybir.dt.float8e4
F32 = mybir.dt.float32

B, S, D = 4, 2048, 1024
NPREV = 8
H = 16
HD = D // H  # 64
BS = B * S
N_CORES = 8
T_CORE = BS // N_CORES  # 1024 tokens per core
P = 128  # partition / token-tile size
C = D // P  # 8 contraction chunks
C2 = C // 2  # 4 fp8 DoubleRow chunks (256-wide contraction each)
N_TILES = T_CORE // P  # 8 token tiles per core
HALF = 512  # matmul free-dim (one PSUM bank)
SCALE = 1.0 / float(np.sqrt(HD))
# fp8 scaling: history x16, Wk/Wv x64 keep e4m3 quantization in the normal
# range; the 16*64 factor is descaled via the exp() scale (k path) and a
# host-side 1/1024 fold into Wo (v path).
H_SCALE = 16.0
W_SCALE = 64.0
KV_DESCALE = 1.0 / (H_SCALE * W_SCALE)
DR = mybir.MatmulPerfMode.DoubleRow

_CACHE = {}


def build_program(
    n_tiles=N_TILES,
    repeat=1,
    attention=True,
    dma_once=False,
    no_tp=False,
    mm_only=False,
):
    """Build the single-core Bass/Tile program (run SPMD on 8 cores).

    repeat>1 wraps the whole computation in an on-device For_i loop —
    used only for timing (amortizes host dispatch overhead away).
    attention=False drops the DVE/ACT attention chain (timing diagnostic
    only — wrong numerics).
    """
    nc = bacc.Bacc("TRN2", debug=False, num_devices=N_CORES)
    t_tok = n_tiles * P

    curP = nc.dram_tensor("curP", [n_tiles, P, C2, 2, P], F8, kind="ExternalInput").ap()
    histP = nc.dram_tensor(
        "histP", [n_tiles, NPREV, P, C2, 2, P], F8, kind="ExternalInput"
    ).ap()
    xres = nc.dram_tensor("xres", [t_tok, D], F32, kind="ExternalInput").ap()
    wq = nc.dram_tensor("wqT", [C2, P, 2, D], F8, kind="ExternalInput").ap()
    wk = nc.dram_tensor("wkT", [C2, P, 2, D], F8, kind="ExternalInput").ap()
    wv = nc.dram_tensor("wvT", [C2, P, 2, D], F8, kind="ExternalInput").ap()
    wo = nc.dram_tensor("woT", [C2, P, 2, D], F8, kind="ExternalInput").ap()
    y = nc.dram_tensor("y", [t_tok, D], F32, kind="ExternalOutput").ap()

    with ExitStack() as ctx:
        tc = ctx.enter_context(tile.TileContext(nc))

        wpool = ctx.enter_context(tc.tile_pool(name="wpool", bufs=1))
        inpool = ctx.enter_context(tc.tile_pool(name="inpool", bufs=2))
        qpool = ctx.enter_context(tc.tile_pool(name="qpool", bufs=2))
        prodpool = ctx.enter_context(tc.tile_pool(name="prodpool", bufs=3))
        spool = ctx.enter_context(tc.tile_pool(name="spool", bufs=2))
        accpool = ctx.enter_context(tc.tile_pool(name="accpool", bufs=2))
        aopool = ctx.enter_context(tc.tile_pool(name="aopool", bufs=2))
        atpool = ctx.enter_context(tc.tile_pool(name="atpool", bufs=2))
        ypool = ctx.enter_context(tc.tile_pool(name="ypool", bufs=2))
        psum = ctx.enter_context(tc.tile_pool(name="psum", bufs=4, space="PSUM"))

        # Resident weights: w_sb[name][p, c2, i, dout] = W?T[c2*256+i*128+p, dout] * W_SCALE
        w_sb = {}
        for name, dram in (("wq", wq), ("wk", wk), ("wv", wv), ("wo", wo)):
            wt = wpool.tile([P, C2, 2, D], F8, name=f"{name}_sb", tag=name)
            nc.sync.dma_start(wt[:], dram.rearrange("c p i d -> p c i d"))
            w_sb[name] = wt
        ident = wpool.tile([P, P], BF16, name="ident", tag="ident")
        make_identity(nc, ident[:])
        if no_tp:
            ident8 = wpool.tile([P, P], F8, name="ident8", tag="ident8")
            nc.scalar.copy(ident8[:], ident[:])

        # state carried from phase A (projections+attention) to phase B
        # (output projection), software-pipelined one tile deep so the
        # tensor engine never waits on the vector engine.
        state = [None] * n_tiles

        dma_cache = {}

        def phase_a(it):
            tok0 = it * P
            if dma_once and it > 0:
                xt, ht, xr = dma_cache[0]
            else:
                xt = inpool.tile([P, C2, 2, P], F8, tag="xt", name="xt")
                nc.sync.dma_start(xt[:], curP[it])
                ht = inpool.tile([P, NPREV, C2, 2, P], F8, tag="ht", name="ht")
                nc.sync.dma_start(ht[:], histP[it].rearrange("n p c i t -> p n c i t"))
                xr = inpool.tile([P, D], F32, tag="xr", name="xr")
                nc.sync.dma_start(xr[:], xres[tok0 : tok0 + P, :])
                dma_cache[0] = (xt, ht, xr)

            # q projection: q[t, dout] accumulated over 4 DoubleRow din chunks
            qp = psum.tile([P, D], F32, tag="mm", name="qp")
            if not mm_only:
                for c in range(C2):
                    for j in range(2):
                        nc.tensor.matmul(
                            qp[:, j * HALF : (j + 1) * HALF],
                            lhsT=xt[:, c],
                            rhs=w_sb["wq"][:, c, :, j * HALF : (j + 1) * HALF],
                            start=(c == 0),
                            stop=(c == C2 - 1),
                            perf_mode=DR,
                        )
                # descale the fp8 q path during the PSUM->SBUF copy
                q_sb = qpool.tile([P, D], F32, tag="q", name="q_sb")
                nc.scalar.mul(q_sb[:], qp[:], KV_DESCALE)
                q_v = q_sb.rearrange("p (h e) -> p h e", e=HD)

            expt = spool.tile([P, NPREV, H], F32, tag="expt", name="expt")
            ssum = spool.tile([P, H], F32, tag="ssum", name="ssum")
            rsum = spool.tile([P, H], F32, tag="rsum", name="rsum")
            acc = accpool.tile([P, H, HD], F32, tag="acc", name="acc")

            for n in range(NPREV):
                kp = psum.tile([P, D], F32, tag="mm", name="kp")
                vp = psum.tile([P, D], F32, tag="mm", name="vp")
                for c in range(C2):
                    st = ht[:, n, c]  # [P, 2, P] fp8 DoubleRow stationary
                    for j in range(2):
                        nc.tensor.matmul(
                            kp[:, j * HALF : (j + 1) * HALF],
                            lhsT=st,
                            rhs=w_sb["wk"][:, c, :, j * HALF : (j + 1) * HALF],
                            start=(c == 0),
                            stop=(c == C2 - 1),
                            perf_mode=DR,
                        )
                        nc.tensor.matmul(
                            vp[:, j * HALF : (j + 1) * HALF],
                            lhsT=st,
                            rhs=w_sb["wv"][:, c, :, j * HALF : (j + 1) * HALF],
                            start=(c == 0),
                            stop=(c == C2 - 1),
                            perf_mode=DR,
                        )

                if not attention or mm_only:
                    continue
                # logits_n[t, h] = sum_e q[t,h,e] * k_n[t,h,e]
                prod = prodpool.tile([P, H, HD], F32, tag="prod", name="prod")
                nc.vector.tensor_mul(
                    prod[:], q_v, kp.rearrange("p (h e) -> p h e", e=HD)
                )
                logit_n = spool.tile([P, H], F32, tag="logit", name="logit_n")
                nc.vector.reduce_sum(logit_n[:], prod[:], axis=mybir.AxisListType.X)
                # exp(scale * logits) on the scalar engine
                nc.scalar.activation(
                    expt[:, n, :],
                    logit_n[:],
                    mybir.ActivationFunctionType.Exp,
                    scale=SCALE * KV_DESCALE,
                )
                # acc += exp_n (broadcast over hd) * v_n
                e_ap = expt[:, n, :]
                e_b = bass.AP(e_ap.tensor, e_ap.offset, e_ap.ap + [[0, HD]])
                v_v = vp.rearrange("p (h e) -> p h e", e=HD)
                if n == 0:
                    nc.vector.tensor_mul(acc[:], v_v, e_b)
                else:
                    avt = prodpool.tile([P, H, HD], F32, tag="prod", name="avt")
                    nc.vector.tensor_mul(avt[:], v_v, e_b)
                    nc.vector.tensor_add(acc[:], acc[:], avt[:])

            if mm_only:
                state[it] = None
                return
            aout = aopool.tile([P, D], BF16, tag="aout", name="aout")
            if attention:
                # softmax denominator and normalization (+ cast to bf16)
                nc.vector.reduce_sum(
                    ssum[:], expt.rearrange("p n h -> p h n"), axis=mybir.AxisListType.X
                )
                nc.vector.reciprocal(rsum[:], ssum[:])
                r_ap = rsum[:]
                r_b = bass.AP(r_ap.tensor, r_ap.offset, r_ap.ap + [[0, HD]])
                nc.vector.tensor_mul(
                    aout.rearrange("p (h e) -> p h e", e=HD), acc[:], r_b
                )
            else:
                nc.scalar.copy(aout[:], qp[:])
            state[it] = (aout, xr, tok0)

        def phase_b(it):
            aout, xr, tok0 = state[it]
            state[it] = None
            # transpose attention output 128x128 on the tensor engine;
            # the PSUM->SBUF copy converts to fp8 with a 1/64 rescale
            # (aout carries x1024 from the fp8 v path -> aoutT carries x16)
            if no_tp:
                ia = ident8[:]
                aoutT = bass.AP(
                    ia.tensor, ia.offset, [ia.ap[0], [0, C2], [0, 2], ia.ap[1]]
                )
            else:
                aoutT = atpool.tile([P, C2, 2, P], F8, tag="aoutT", name="aoutT")
                for g in range(2):
                    tp = psum.tile([P, 4 * P], BF16, tag="mm", name="tp")
                    for cc in range(4):
                        c = g * 4 + cc
                        nc.tensor.transpose(
                            tp[:, cc * P : (cc + 1) * P],
                            aout[:, c * P : (c + 1) * P],
                            ident[:],
                        )
                    nc.scalar.mul(
                        aoutT[:, g * 2 : (g + 1) * 2].rearrange("p c i t -> p (c i t)"),
                        tp[:],
                        1.0 / 64.0,
                    )
            # output projection (fp8 DoubleRow) + descale + residual
            yp = psum.tile([P, D], F32, tag="mm", name="yp")
            for c in range(C2):
                for j in range(2):
                    nc.tensor.matmul(
                        yp[:, j * HALF : (j + 1) * HALF],
                        lhsT=aoutT[:, c],
                        rhs=w_sb["wo"][:, c, :, j * HALF : (j + 1) * HALF],
                        start=(c == 0),
                        stop=(c == C2 - 1),
                        perf_mode=DR,
                    )
            y_sb = ypool.tile([P, D], F32, tag="ysb", name="y_sb")
            nc.scalar.mul(y_sb[:], yp[:], KV_DESCALE)
            nc.vector.tensor_add(y_sb[:], y_sb[:], xr[:])
            nc.sync.dma_start(y[tok0 : tok0 + P, :], y_sb[:])

        def whole_body():
            for it in range(n_tiles + 1):
                if it < n_tiles:
                    phase_a(it)
                if it >= 1 and not mm_only:
                    phase_b(it - 1)

        if repeat == 1:
            whole_body()
        else:
            with tc.For_i(0, repeat, 1):
                whole_body()

    nc.compile()
    return nc


def prep_inputs(current, history, Wq, Wk, Wv, Wo, n_cores=N_CORES):
    """Host-side shard + layout prep.  Returns per-core input maps."""
    bf16 = ml_dtypes.bfloat16
    f8 = ml_dtypes.float8_e4m3  # TRN FP8_EXP4-compatible (max +-240)
    cur = np.ascontiguousarray(current.reshape(BS, D)).astype(np.float32)
    hist = history.reshape(BS, NPREV, D)

    n_tiles_total = BS // P
    # curP[tile, p, c2, i, t] = cur[tile*128 + t, c2*256 + i*128 + p] * 16
    cur_f8 = (cur * H_SCALE).astype(f8)
    curP = np.ascontiguousarray(
        cur_f8.reshape(n_tiles_total, P, C2, 2, P).transpose(0, 4, 2, 3, 1)
    )
    # histP[tile, n, p, c2, i, t] = hist[tile*128 + t, n, c2*256 + i*128 + p] * 16
    hist_f8 = (hist.astype(np.float32) * H_SCALE).astype(f8)
    histP = np.ascontiguousarray(
        hist_f8.reshape(n_tiles_total, P, NPREV, C2, 2, P).transpose(0, 2, 5, 3, 4, 1)
    )

    def wprep8(w):
        # w?T8[c2, p, i, dout] = W.T[c2*256 + i*128 + p, dout] * 64
        wt = (w.T.astype(np.float32) * W_SCALE).astype(f8)
        return np.ascontiguousarray(wt.reshape(C2, 2, P, D).transpose(0, 2, 1, 3))

    wqT, wkT, wvT, woT = (wprep8(w) for w in (Wq, Wk, Wv, Wo))

    tiles_per_core = n_tiles_total // n_cores
    in_maps = []
    for ci in range(n_cores):
        t0 = ci * tiles_per_core
        sl = slice(ci * T_CORE, (ci + 1) * T_CORE)
        in_maps.append(
            {
                "curP": np.ascontiguousarray(curP[t0 : t0 + tiles_per_core]),
                "histP": np.ascontiguousarray(histP[t0 : t0 + tiles_per_core]),
                "xres": np.ascontiguousarray(cur[sl]),
                "wqT": wqT,
                "wkT": wkT,
                "wvT": wvT,
                "woT": woT,
            }
        )
    return in_maps


def kernel(current, history, Wq, Wk, Wv, Wo):
    from concourse.bass_utils import run_bass_kernel_spmd

    if "nc" not in _CACHE:
        _CACHE["nc"] = build_program(N_TILES)
    nc = _CACHE["nc"]

    in_maps = prep_inputs(current, history, Wq, Wk, Wv, Wo)
    results = run_bass_kernel_spmd(nc, in_maps, core_ids=list(range(N_CORES))).results
    y = np.concatenate([results[ci]["y"] for ci in range(N_CORES)], axis=0)
    return y.reshape(B, S, D).astype(np.float32)

